# revision 2
# baseline (speedup 1.0000x reference)
"""Trainium2 Bass kernel for nn_KVMem (scatter_memory attention-to-memory).

Computation (per reference):
  q = h.reshape(B,S,8,128); k = keys_w.reshape(32768,8,128)
  w = softmax(einsum('bshd,zhd->bshz', q, k), axis=z)
  out = einsum('bshz,hdz->bshd', w, values_w.reshape(8,128,32768))

Strategy: shard the memory axis z (32768) across 8 cores (4096 each).
All data in fp16. Per core, per head, for each 128-z tile:
  S[z,tok]  = K_tile.T @ Q           (TensorE, stationary K-tile, streams Q)
  P[z,tok]  = exp(S)                 (ScalarE, PSUM->SBUF fp16, 2-ztile instrs)
  O[d,tok] += V_tile.T @ P           (TensorE, stationary V-tile, streams P)
  D[z%128,tok] += P                  (VectorE/GpSimd elementwise accumulate)
The denominator sum_z exp(S) per token = partition-sum of D, done on HOST
along with the cross-core reduction of (O, D) and the final division.

Both matmuls stream 512-col moving operands (stationary loads hidden via
FWL + the PE reorder window), so TensorE runs near its streaming floor.
The exp work is issued as [128, 2048] activations over a 3-slot PSUM
scores ring (6 banks) to amortize ScalarE per-instruction overhead; the
O accumulator [128,1024] occupies the remaining 2 banks.
"""

import sys

sys.path.insert(0, "/opt/trn_rl_repo")

import numpy as np
import ml_dtypes

NCORES = 8
MEMDIM, MEMSIZE, NHEADS = 1024, 32768, 8
B, S = 2, 512
TOK = B * S  # 1024
HD = MEMDIM // NHEADS  # 128
ZL = MEMSIZE // NCORES  # 4096 z per core
ZT = ZL // 128  # 32 z-tiles per core (per head)
NPAIR = ZT // 2  # 16 z-tile pairs per head

# head-local pair indices whose D-accumulation runs on GpSimd (rest on DVE)
GP_PAIRS = frozenset({3, 6, 9, 12, 15})

_compiled = None


def _build():
    import concourse.bass as bass
    import concourse.tile as tile
    from concourse import bacc, mybir

    nc = bacc.Bacc(
        "TRN2", target_bir_lowering=False, debug=False, num_devices=NCORES
    )
    fp16 = mybir.dt.float16
    f32 = mybir.dt.float32
    ALU = mybir.AluOpType

    qT = nc.dram_tensor("qT", [128, NHEADS * TOK], fp16, kind="ExternalInput").ap()
    kT = nc.dram_tensor("kT", [128, NHEADS * ZL], fp16, kind="ExternalInput").ap()
    vT = nc.dram_tensor(
        "vT", [128, NHEADS * ZT * HD], fp16, kind="ExternalInput"
    ).ap()
    o_out = nc.dram_tensor(
        "o_out", [128, NHEADS * TOK], f32, kind="ExternalOutput"
    ).ap()
    d_out = nc.dram_tensor(
        "d_out", [128, NHEADS * TOK], fp16, kind="ExternalOutput"
    ).ap()
    d2_out = nc.dram_tensor(
        "d2_out", [128, NHEADS * TOK], fp16, kind="ExternalOutput"
    ).ap()

    with tile.TileContext(nc) as tc:
        with (
            tc.tile_pool(name="const", bufs=1) as cpool,
            tc.tile_pool(name="p", bufs=3) as ppool,
            tc.tile_pool(name="d", bufs=2) as dpool,
            tc.tile_pool(name="osb", bufs=2) as opool,
            tc.tile_pool(name="ps", bufs=1, space=bass.MemorySpace.PSUM) as pspool,
        ):
            q_sb = cpool.tile([128, NHEADS * TOK], fp16, tag="q", name="q_sb")
            k_sb = cpool.tile([128, NHEADS * ZL], fp16, tag="k", name="k_sb")
            v_sb = cpool.tile(
                [128, NHEADS * ZT * HD], fp16, tag="v", name="v_sb"
            )

            # 6 banks of scores ring (3 slots x [128,1024] f32) + 2 banks O
            s_all = pspool.tile([128, 3 * TOK], f32, tag="s", name="s_all")
            o_ps = pspool.tile([128, TOK], f32, tag="o", name="o_ps")

            def load_head(h, nchunk):
                nc.sync.dma_start(
                    q_sb[:, h * TOK : (h + 1) * TOK], qT[:, h * TOK : (h + 1) * TOK]
                )
                for ch in range(nchunk):
                    zlo, zhi = ch * ZL // nchunk, (ch + 1) * ZL // nchunk
                    nc.sync.dma_start(
                        k_sb[:, h * ZL + zlo : h * ZL + zhi],
                        kT[:, h * ZL + zlo : h * ZL + zhi],
                    )
                    alo, ahi = zlo // 128 * HD, zhi // 128 * HD
                    nc.sync.dma_start(
                        v_sb[:, h * ZT * HD + alo : h * ZT * HD + ahi],
                        vT[:, h * ZT * HD + alo : h * ZT * HD + ahi],
                    )

            load_head(0, 8)
            load_head(1, 2)

            for h in range(NHEADS):
                if h + 2 < NHEADS:
                    load_head(h + 2, 2)
                d_sb = dpool.tile([128, TOK], fp16, tag="d", name="d_sb")
                d2_sb = dpool.tile([128, TOK], fp16, tag="d2", name="d2_sb")
                dve_init = gp_init = True
                for pl in range(NPAIR):
                    gpair = h * NPAIR + pl
                    slot = [(2 * gpair) % 3, (2 * gpair + 1) % 3]
                    for t in range(2):
                        zt = 2 * pl + t
                        for j in range(2):
                            nc.tensor.matmul(
                                s_all[
                                    :,
                                    slot[t] * TOK + j * 512 : slot[t] * TOK
                                    + (j + 1) * 512,
                                ],
                                k_sb[:, h * ZL + zt * 128 : h * ZL + (zt + 1) * 128],
                                q_sb[:, h * TOK + j * 512 : h * TOK + (j + 1) * 512],
                            )
                    p_sb = ppool.tile([128, 2 * TOK], fp16, tag="p", name="p_sb")
                    if slot[0] == 2:  # wrap pair (slots 2,0): 2D strided src
                        src = s_all.rearrange("p (s t) -> p s t", s=3)[:, 2::-2, :]
                        dst = p_sb.rearrange("p (s t) -> p s t", s=2)
                        nc.scalar.activation(
                            dst, src, mybir.ActivationFunctionType.Exp
                        )
                    else:
                        lo = slot[0] * TOK
                        nc.scalar.activation(
                            p_sb[:],
                            s_all[:, lo : lo + 2 * TOK],
                            mybir.ActivationFunctionType.Exp,
                        )
                    for t in range(2):
                        zt = 2 * pl + t
                        for j in range(2):
                            nc.tensor.matmul(
                                o_ps[:, j * 512 : (j + 1) * 512],
                                v_sb[
                                    :,
                                    (h * ZT + zt) * HD : (h * ZT + zt + 1) * HD,
                                ],
                                p_sb[:, t * TOK + j * 512 : t * TOK + (j + 1) * 512],
                                start=(zt == 0),
                                stop=(zt == ZT - 1),
                            )
                    pa, pb = p_sb[:, 0:TOK], p_sb[:, TOK : 2 * TOK]
                    if pl in GP_PAIRS:
                        if gp_init:
                            nc.gpsimd.tensor_tensor(d2_sb[:], pa, pb, ALU.add)
                            gp_init = False
                        else:
                            nc.gpsimd.tensor_tensor(d2_sb[:], d2_sb[:], pa, ALU.add)
                            nc.gpsimd.tensor_tensor(d2_sb[:], d2_sb[:], pb, ALU.add)
                    else:
                        if dve_init:
                            nc.vector.tensor_tensor(d_sb[:], pa, pb, ALU.add)
                            dve_init = False
                        else:
                            nc.vector.tensor_tensor(d_sb[:], d_sb[:], pa, ALU.add)
                            nc.vector.tensor_tensor(d_sb[:], d_sb[:], pb, ALU.add)
                out_sb = opool.tile([128, TOK], f32, tag="osb", name="out_sb")
                nc.vector.tensor_copy(out_sb[:], o_ps[:])
                nc.sync.dma_start(o_out[:, h * TOK : (h + 1) * TOK], out_sb[:])
                nc.sync.dma_start(d_out[:, h * TOK : (h + 1) * TOK], d_sb[:])
                nc.sync.dma_start(d2_out[:, h * TOK : (h + 1) * TOK], d2_sb[:])

    nc.compile()
    return nc


def _shard_inputs(h, keys_w, values_w):
    f16 = ml_dtypes.float16 if hasattr(ml_dtypes, "float16") else np.float16
    hh = h.reshape(TOK, NHEADS, HD)
    qTf = np.ascontiguousarray(hh.transpose(2, 1, 0).reshape(128, NHEADS * TOK))
    qTf = qTf.astype(np.float16)
    in_maps = []
    for c in range(NCORES):
        ks = keys_w[c * ZL : (c + 1) * ZL]  # [ZL, MEMDIM]
        kTc = np.ascontiguousarray(
            ks.reshape(ZL, NHEADS, HD).transpose(2, 1, 0).reshape(128, NHEADS * ZL)
        ).astype(np.float16)
        vs = values_w[:, c * ZL : (c + 1) * ZL]  # [MEMDIM, ZL]
        v4 = vs.reshape(NHEADS, HD, ZT, 128)  # [h, d, zt, p]
        vTc = np.ascontiguousarray(
            v4.transpose(3, 0, 2, 1).reshape(128, NHEADS * ZT * HD)
        ).astype(np.float16)
        in_maps.append({"qT": qTf, "kT": kTc, "vT": vTc})
    return in_maps


def _combine(results):
    o_acc = np.zeros((128, NHEADS, TOK), np.float64)
    den = np.zeros((NHEADS, TOK), np.float64)
    for r in results:
        o_acc += r["o_out"].reshape(128, NHEADS, TOK).astype(np.float64)
        den += r["d_out"].reshape(128, NHEADS, TOK).astype(np.float64).sum(axis=0)
        den += r["d2_out"].reshape(128, NHEADS, TOK).astype(np.float64).sum(axis=0)
    res = o_acc / den[None, :, :]  # [d, h, t]
    res = res.transpose(2, 1, 0)  # [t, h, d]
    return np.ascontiguousarray(
        res.reshape(TOK, MEMDIM).reshape(B, S, MEMDIM).astype(np.float32)
    )


def kernel(h, keys_w, values_w, _trace=False, _tmpdir=None):
    global _compiled
    if _compiled is None:
        _compiled = _build()
    from concourse import bass_utils

    in_maps = _shard_inputs(
        np.asarray(h, dtype=np.float32),
        np.asarray(keys_w, dtype=np.float32),
        np.asarray(values_w, dtype=np.float32),
    )
    res = bass_utils.run_bass_kernel_spmd(
        _compiled,
        in_maps,
        core_ids=list(range(NCORES)),
        trace=_trace,
        tmpdir=_tmpdir,
    )
    out = _combine(res.results)
    if _trace:
        return out, res
    return out


# revision 9
# speedup vs baseline: 1.5559x; 1.5559x over previous
"""Trainium2 Bass kernel for nn_KVMem (scatter_memory attention-to-memory).

Computation (per reference):
  q = h.reshape(B,S,8,128); k = keys_w.reshape(32768,8,128)
  w = softmax(einsum('bshd,zhd->bshz', q, k), axis=z)
  out = einsum('bshz,hdz->bshd', w, values_w.reshape(8,128,32768))

Strategy: shard the memory axis z (32768) across 8 cores (4096 each).
All data in fp16. Per core, per head, for each 128-z tile:
  S[z,tok]  = K_tile.T @ Q           (TensorE, stationary K-tile, streams Q)
  P[z,tok]  = exp(S)                 (ScalarE, PSUM->SBUF fp16, 2-ztile instrs)
  O[d,tok] += V_tile.T @ P           (TensorE, stationary V-tile, streams P)
  D[z%128,tok] += P                  (VectorE/GpSimd elementwise accumulate)
The denominator sum_z exp(S) per token = partition-sum of D, done on HOST
along with the cross-core reduction of (O, D) and the final division.

Both matmuls stream 512-col moving operands (stationary loads hidden via
FWL + the PE reorder window), so TensorE runs near its streaming floor.
The exp work is issued as [128, 2048] activations over a 3-slot PSUM
scores ring (6 banks) to amortize ScalarE per-instruction overhead; the
O accumulator [128,1024] occupies the remaining 2 banks.
"""

import sys

sys.path.insert(0, "/opt/trn_rl_repo")

import numpy as np
import ml_dtypes

NCORES = 8
MEMDIM, MEMSIZE, NHEADS = 1024, 32768, 8
B, S = 2, 512
TOK = B * S  # 1024
HD = MEMDIM // NHEADS  # 128
ZL = MEMSIZE // NCORES  # 4096 z per core
ZT = ZL // 128  # 32 z-tiles per core (per head)
NPAIR = ZT // 2  # 16 z-tile pairs per head

# head-local pair indices whose D-accumulation runs on GpSimd (rest on DVE)
GP_PAIRS = frozenset()

_compiled = None


def _build():
    import concourse.bass as bass
    import concourse.tile as tile
    from concourse import bacc, mybir

    nc = bacc.Bacc(
        "TRN2", target_bir_lowering=False, debug=False, num_devices=NCORES
    )
    fp16 = mybir.dt.float16
    f32 = mybir.dt.float32
    ALU = mybir.AluOpType

    qT = nc.dram_tensor("qT", [128, NHEADS * TOK], fp16, kind="ExternalInput").ap()
    kT = nc.dram_tensor("kT", [128, NHEADS * ZL], fp16, kind="ExternalInput").ap()
    vT = nc.dram_tensor(
        "vT", [128, NHEADS * ZT * HD], fp16, kind="ExternalInput"
    ).ap()
    o_out = nc.dram_tensor(
        "o_out", [128, NHEADS * TOK], f32, kind="ExternalOutput"
    ).ap()
    d_out = nc.dram_tensor(
        "d_out", [128, NHEADS * TOK], fp16, kind="ExternalOutput"
    ).ap()
    d2_out = (
        nc.dram_tensor(
            "d2_out", [128, NHEADS * TOK], fp16, kind="ExternalOutput"
        ).ap()
        if GP_PAIRS
        else None
    )

    with tile.TileContext(nc) as tc:
        with (
            tc.tile_pool(name="const", bufs=1) as cpool,
            tc.tile_pool(name="p", bufs=5) as ppool,
            tc.tile_pool(name="d", bufs=2) as dpool,
            tc.tile_pool(name="osb", bufs=2) as opool,
            tc.tile_pool(name="ps", bufs=1, space=bass.MemorySpace.PSUM) as pspool,
        ):
            q_sb = cpool.tile([128, NHEADS * TOK], fp16, tag="q", name="q_sb")
            k_sb = cpool.tile([128, NHEADS * ZL], fp16, tag="k", name="k_sb")
            v_sb = cpool.tile(
                [128, NHEADS * ZT * HD], fp16, tag="v", name="v_sb"
            )

            # 6 banks of scores ring (3 slots x [128,1024] f32) + 2 banks O
            s_all = pspool.tile([128, 3 * TOK], f32, tag="s", name="s_all")
            o_ps = pspool.tile([128, TOK], f32, tag="o", name="o_ps")

            def load_head(h, nchunk):
                nc.sync.dma_start(
                    q_sb[:, h * TOK : (h + 1) * TOK], qT[:, h * TOK : (h + 1) * TOK]
                )
                for ch in range(nchunk):
                    zlo, zhi = ch * ZL // nchunk, (ch + 1) * ZL // nchunk
                    nc.sync.dma_start(
                        k_sb[:, h * ZL + zlo : h * ZL + zhi],
                        kT[:, h * ZL + zlo : h * ZL + zhi],
                    )
                    alo, ahi = zlo // 128 * HD, zhi // 128 * HD
                    nc.sync.dma_start(
                        v_sb[:, h * ZT * HD + alo : h * ZT * HD + ahi],
                        vT[:, h * ZT * HD + alo : h * ZT * HD + ahi],
                    )

            load_head(0, 8)
            load_head(1, 2)

            for h in range(NHEADS):
                if h + 2 < NHEADS:
                    load_head(h + 2, 2)
                d_sb = dpool.tile([128, TOK], fp16, tag="d", name="d_sb")
                d2_sb = (
                    dpool.tile([128, TOK], fp16, tag="d2", name="d2_sb")
                    if GP_PAIRS
                    else None
                )
                state = {"dve_init": True, "gp_init": True}

                def consume_pair(pl, p_sb):
                    # V-matmuls + D-accumulation for an exp'd pair (deferred
                    # one pair so PE always has score-work ahead of the exp
                    # dependency).
                    for t in range(2):
                        zt = 2 * pl + t
                        for j in range(2):
                            nc.tensor.matmul(
                                o_ps[:, j * 512 : (j + 1) * 512],
                                v_sb[
                                    :,
                                    (h * ZT + zt) * HD : (h * ZT + zt + 1) * HD,
                                ],
                                p_sb[:, t * TOK + j * 512 : t * TOK + (j + 1) * 512],
                                start=(zt == 0),
                                stop=(zt == ZT - 1),
                            )
                    pa, pb = p_sb[:, 0:TOK], p_sb[:, TOK : 2 * TOK]
                    if pl in GP_PAIRS:
                        if state["gp_init"]:
                            nc.gpsimd.tensor_tensor(d2_sb[:], pa, pb, ALU.add)
                            state["gp_init"] = False
                        else:
                            nc.gpsimd.tensor_tensor(d2_sb[:], d2_sb[:], pa, ALU.add)
                            nc.gpsimd.tensor_tensor(d2_sb[:], d2_sb[:], pb, ALU.add)
                    else:
                        if state["dve_init"]:
                            nc.vector.tensor_tensor(d_sb[:], pa, pb, ALU.add)
                            state["dve_init"] = False
                        else:
                            nc.vector.tensor_tensor(d_sb[:], d_sb[:], pa, ALU.add)
                            nc.vector.tensor_tensor(d_sb[:], d_sb[:], pb, ALU.add)

                pending = None
                for pl in range(NPAIR):
                    gpair = h * NPAIR + pl
                    slot = [(2 * gpair) % 3, (2 * gpair + 1) % 3]
                    for t in range(2):
                        zt = 2 * pl + t
                        for j in range(2):
                            nc.tensor.matmul(
                                s_all[
                                    :,
                                    slot[t] * TOK + j * 512 : slot[t] * TOK
                                    + (j + 1) * 512,
                                ],
                                k_sb[:, h * ZL + zt * 128 : h * ZL + (zt + 1) * 128],
                                q_sb[:, h * TOK + j * 512 : h * TOK + (j + 1) * 512],
                            )
                    p_sb = ppool.tile([128, 2 * TOK], fp16, tag="p", name="p_sb")
                    if slot[0] == 2:  # wrap pair (slots 2,0): 2D strided src
                        src = s_all.rearrange("p (s t) -> p s t", s=3)[:, 2::-2, :]
                        dst = p_sb.rearrange("p (s t) -> p s t", s=2)
                        nc.scalar.activation(
                            dst, src, mybir.ActivationFunctionType.Exp
                        )
                    else:
                        lo = slot[0] * TOK
                        nc.scalar.activation(
                            p_sb[:],
                            s_all[:, lo : lo + 2 * TOK],
                            mybir.ActivationFunctionType.Exp,
                        )
                    if pending is not None:
                        consume_pair(*pending)
                    pending = (pl, p_sb)
                consume_pair(*pending)
                out_sb = opool.tile([128, TOK], f32, tag="osb", name="out_sb")
                nc.vector.tensor_copy(out_sb[:], o_ps[:])
                nc.sync.dma_start(o_out[:, h * TOK : (h + 1) * TOK], out_sb[:])
                nc.sync.dma_start(d_out[:, h * TOK : (h + 1) * TOK], d_sb[:])
                if GP_PAIRS:
                    nc.sync.dma_start(
                        d2_out[:, h * TOK : (h + 1) * TOK], d2_sb[:]
                    )

    nc.compile()
    return nc


def _shard_inputs(h, keys_w, values_w):
    f16 = ml_dtypes.float16 if hasattr(ml_dtypes, "float16") else np.float16
    hh = h.reshape(TOK, NHEADS, HD)
    qTf = np.ascontiguousarray(hh.transpose(2, 1, 0).reshape(128, NHEADS * TOK))
    qTf = qTf.astype(np.float16)
    in_maps = []
    for c in range(NCORES):
        ks = keys_w[c * ZL : (c + 1) * ZL]  # [ZL, MEMDIM]
        kTc = np.ascontiguousarray(
            ks.reshape(ZL, NHEADS, HD).transpose(2, 1, 0).reshape(128, NHEADS * ZL)
        ).astype(np.float16)
        vs = values_w[:, c * ZL : (c + 1) * ZL]  # [MEMDIM, ZL]
        v4 = vs.reshape(NHEADS, HD, ZT, 128)  # [h, d, zt, p]
        vTc = np.ascontiguousarray(
            v4.transpose(3, 0, 2, 1).reshape(128, NHEADS * ZT * HD)
        ).astype(np.float16)
        in_maps.append({"qT": qTf, "kT": kTc, "vT": vTc})
    return in_maps


def _combine(results):
    o_acc = np.zeros((128, NHEADS, TOK), np.float64)
    den = np.zeros((NHEADS, TOK), np.float64)
    for r in results:
        o_acc += r["o_out"].reshape(128, NHEADS, TOK).astype(np.float64)
        den += r["d_out"].reshape(128, NHEADS, TOK).astype(np.float64).sum(axis=0)
        if "d2_out" in r:
            den += (
                r["d2_out"].reshape(128, NHEADS, TOK).astype(np.float64).sum(axis=0)
            )
    res = o_acc / den[None, :, :]  # [d, h, t]
    res = res.transpose(2, 1, 0)  # [t, h, d]
    return np.ascontiguousarray(
        res.reshape(TOK, MEMDIM).reshape(B, S, MEMDIM).astype(np.float32)
    )


def kernel(h, keys_w, values_w, _trace=False, _tmpdir=None):
    global _compiled
    if _compiled is None:
        _compiled = _build()
    from concourse import bass_utils

    in_maps = _shard_inputs(
        np.asarray(h, dtype=np.float32),
        np.asarray(keys_w, dtype=np.float32),
        np.asarray(values_w, dtype=np.float32),
    )
    res = bass_utils.run_bass_kernel_spmd(
        _compiled,
        in_maps,
        core_ids=list(range(NCORES)),
        trace=_trace,
        tmpdir=_tmpdir,
    )
    out = _combine(res.results)
    if _trace:
        return out, res
    return out
